# revision 1
# baseline (speedup 1.0000x reference)
"""GPT2 (L=12, D=1024, H=16, S=512, B=4, V=16386) on 8 trn2 NeuronCores.

Scheme: token-data-parallel. Each core owns 256 tokens (2 causal-balanced
blocks of 128 within one batch; pair cores 2c/2c+1 split batch c).
Per layer: LN1(own) -> pair-AllGather of x_ln (bf16) -> k/v for the whole
batch computed locally from the gathered x_ln -> attention for own q tokens
(causality via per-core 0/1 mask inputs, keeps the SPMD program uniform)
-> proj/LN2/FC/gelu/MLP on own tokens only. lm_head token-sharded.

Layout: activations feature-major [D, tok] (tokens on the free dim), v
token-major; LN affine folded into weights host-side; weights bf16, PSUM
and residual stream f32.
"""

import os
import numpy as np
import ml_dtypes

# ---- static config (must match reference.py) ----
L = 12
D = 1024
H = 16
DH = 64
S = 512
B = 4
V = 16386
EPS = 1e-5
SCALE = 1.0 / 8.0  # 1/sqrt(DH)

P = 128
KT = D // P           # 8 k-tiles over D
TOK = 256             # own tokens per core
SB = 512              # batch tokens (kv length)
FF = 4096
FFT = FF // P         # 32
VPAD = 16896          # 33 * 512
NV = VPAD // 512      # 33

BF = ml_dtypes.bfloat16

# rank-order kv column blocks: chunk cc -> seq block id
BB = [0, 3, 1, 2]
# core parity -> owned q blocks
QBLOCKS = {0: (0, 3), 1: (1, 2)}

N_LAYERS = int(os.environ.get("GPT2_N_LAYERS", str(L)))


def _build(n_layers):
    from concourse import bacc, bass, mybir
    import concourse.tile as tile

    F32 = mybir.dt.float32
    BD = mybir.dt.bfloat16
    AF = mybir.ActivationFunctionType
    OP = mybir.AluOpType

    nc = bacc.Bacc("TRN2", target_bir_lowering=False, debug=False,
                   num_devices=8)

    # ---- kernel I/O ----
    h0T = nc.dram_tensor("h0T", [D, TOK], F32, kind="ExternalInput").ap()
    wqkv = nc.dram_tensor("wqkv", [n_layers * D, 3 * D], BD,
                          kind="ExternalInput").ap()
    wproj = nc.dram_tensor("wproj", [n_layers * D, D], BD,
                           kind="ExternalInput").ap()
    wfc = nc.dram_tensor("wfc", [n_layers * D, FF], BD,
                         kind="ExternalInput").ap()
    wmlp = nc.dram_tensor("wmlp", [n_layers * FF, D], BD,
                          kind="ExternalInput").ap()
    whead = nc.dram_tensor("whead", [NV * KT * P, 512], BD,
                           kind="ExternalInput").ap()
    # biases per layer, laid out [n_layers*128, 72]:
    #   cols 0:24 qkv (24 ptiles of 3072), 24:32 proj, 32:64 fc, 64:72 mlp
    bvec = nc.dram_tensor("bvec", [n_layers * P, 72], F32,
                          kind="ExternalInput").ap()
    # v bias as a row (applied via K=1 matmul): [n_layers, 1024] bf16
    bvrow = nc.dram_tensor("bvrow", [n_layers, D], BD,
                           kind="ExternalInput").ap()
    masks = nc.dram_tensor("masks", [4 * P, TOK], BD,
                           kind="ExternalInput").ap()
    out = nc.dram_tensor("out", [TOK, VPAD], F32, kind="ExternalOutput").ap()

    # internal DRAM for the per-layer pair all-gather
    agin = []
    agout = []
    for l in range(n_layers):
        agin.append(nc.dram_tensor(f"agin{l}", [P, KT * TOK], BD,
                                   kind="Internal").ap())
        agout.append(nc.dram_tensor(f"agout{l}", [2 * P, KT * TOK], BD,
                                    kind="Internal").ap())

    from contextlib import ExitStack

    with tile.TileContext(nc) as tc:
        with ExitStack() as ctx:
            consts = ctx.enter_context(tc.tile_pool(name="consts", bufs=1))
            resid = ctx.enter_context(tc.tile_pool(name="resid", bufs=2))
            wbig = ctx.enter_context(tc.tile_pool(name="wbig", bufs=8))
            xbp = ctx.enter_context(tc.tile_pool(name="xb", bufs=1))
            xlnp = ctx.enter_context(tc.tile_pool(name="xln", bufs=2))
            xagp = ctx.enter_context(tc.tile_pool(name="xag", bufs=1))
            qtp = ctx.enter_context(tc.tile_pool(name="qt", bufs=1))
            ktp = ctx.enter_context(tc.tile_pool(name="kt", bufs=1))
            vfp = ctx.enter_context(tc.tile_pool(name="vf", bufs=1))
            ggp = ctx.enter_context(tc.tile_pool(name="gg", bufs=1))
            oop = ctx.enter_context(tc.tile_pool(name="oo", bufs=1))
            exp_pool = ctx.enter_context(tc.tile_pool(name="ex", bufs=6))
            smp = ctx.enter_context(tc.tile_pool(name="sm", bufs=4))
            scr = ctx.enter_context(tc.tile_pool(name="sc", bufs=2))
            obp = ctx.enter_context(tc.tile_pool(name="ob", bufs=1))
            pmm = ctx.enter_context(
                tc.tile_pool(name="pmm", bufs=2, space="PSUM"))
            pscp = ctx.enter_context(
                tc.tile_pool(name="psc", bufs=4, space="PSUM"))
            pop = ctx.enter_context(
                tc.tile_pool(name="po", bufs=2, space="PSUM"))
            # ---- constants ----
            ones_col = consts.tile([P, 1], BD)
            nc.vector.memset(ones_col, 1.0)
            ones_row = consts.tile([1, P], BD)
            nc.vector.memset(ones_row, 1.0)
            eps_sb = consts.tile([1, 1], F32)
            nc.vector.memset(eps_sb, EPS)
            mask_sb = consts.tile([P, 4, TOK], BD)
            nc.sync.dma_start(
                out=mask_sb,
                in_=masks.rearrange("(c p) t -> p c t", c=4))
            ball = consts.tile([P, n_layers, 72], F32)
            nc.sync.dma_start(
                out=ball,
                in_=bvec.rearrange("(l p) j -> p l j", l=n_layers))

            # residual stream (feature-major, f32), ping-pong
            h_a = resid.tile([P, KT, TOK], F32)
            h_b = resid.tile([P, KT, TOK], F32)
            nc.sync.dma_start(out=h_a,
                              in_=h0T.rearrange("(k p) t -> p k t", k=KT))

            def emit_ln(xb, ntok, x_ln):
                """xb: [P, KT, ntok] bf16 SBUF -> x_ln (normalized, bf16)."""
                ps = pop.tile([1, ntok], F32, tag="po")
                pq = pop.tile([1, ntok], F32, tag="po")
                sq = scr.tile([P, KT, TOK], BD, tag="scr")
                nc.vector.tensor_mul(sq[:, :, :ntok], xb, xb)
                for kk in range(KT):
                    nc.tensor.matmul(ps, ones_col, xb[:, kk, :],
                                     start=(kk == 0), stop=(kk == KT - 1))
                for kk in range(KT):
                    nc.tensor.matmul(pq, ones_col, sq[:, kk, :ntok],
                                     start=(kk == 0), stop=(kk == KT - 1))
                mu_f = smp.tile([1, TOK], F32, tag="sm")
                mu_b = smp.tile([1, TOK], BD, tag="smb")
                e2 = smp.tile([1, TOK], F32, tag="sm")
                m2 = smp.tile([1, TOK], F32, tag="sm")
                var = smp.tile([1, TOK], F32, tag="sm")
                sd = smp.tile([1, TOK], F32, tag="sm")
                inv = smp.tile([1, TOK], F32, tag="sm")
                inv_b = smp.tile([1, TOK], BD, tag="smb")
                nc.vector.tensor_scalar_mul(mu_f[:, :ntok], ps, 1.0 / D)
                nc.vector.tensor_scalar_mul(mu_b[:, :ntok], ps, 1.0 / D)
                nc.vector.tensor_scalar_mul(e2[:, :ntok], pq, 1.0 / D)
                nc.vector.tensor_mul(m2[:, :ntok], mu_f[:, :ntok],
                                     mu_f[:, :ntok])
                nc.vector.tensor_sub(var[:, :ntok], e2[:, :ntok],
                                     m2[:, :ntok])
                nc.scalar.activation(sd[:, :ntok], var[:, :ntok],
                                     AF.Sqrt, bias=eps_sb)
                nc.vector.reciprocal(inv[:, :ntok], sd[:, :ntok])
                nc.vector.tensor_copy(out=inv_b[:, :ntok],
                                      in_=inv[:, :ntok])
                pmu = pop.tile([P, ntok], F32, tag="po")
                pa = pop.tile([P, ntok], F32, tag="po")
                nc.tensor.matmul(pmu, ones_row, mu_b[:, :ntok],
                                 start=True, stop=True)
                nc.tensor.matmul(pa, ones_row, inv_b[:, :ntok],
                                 start=True, stop=True)
                mu_sb = scr.tile([P, TOK], BD, tag="scr2")
                a_sb = scr.tile([P, TOK], BD, tag="scr2")
                nc.scalar.copy(mu_sb[:, :ntok], pmu)
                nc.scalar.copy(a_sb[:, :ntok], pa)
                for kk in range(KT):
                    nc.vector.tensor_sub(sq[:, kk, :ntok], xb[:, kk, :],
                                         mu_sb[:, :ntok])
                    nc.vector.tensor_mul(x_ln[:, kk, :], sq[:, kk, :ntok],
                                         a_sb[:, :ntok])

            for l in range(n_layers):
                lb = ball[:, l, :]
                # ---- x_b = bf16(h) ; LN1 on own tokens ----
                x_b = xbp.tile([P, KT, TOK], BD)
                nc.scalar.copy(x_b, h_a)
                x_ln = xlnp.tile([P, KT, TOK], BD)
                emit_ln(x_b, TOK, x_ln)

                # ---- all-gather x_ln across the pair ----
                nc.sync.dma_start(
                    out=agin[l].rearrange("p (k t) -> p k t", k=KT),
                    in_=x_ln)
                nc.gpsimd.collective_compute(
                    "AllGather", mybir.AluOpType.bypass,
                    replica_groups=[[0, 1], [2, 3], [4, 5], [6, 7]],
                    ins=[agin[l]],
                    outs=[agout[l]],
                )
                x_ag = xagp.tile([P, KT, SB], BD)
                for s in range(2):
                    nc.sync.dma_start(
                        out=x_ag[:, :, s * TOK:(s + 1) * TOK],
                        in_=agout[l][s * P:(s + 1) * P, :].rearrange(
                            "p (k t) -> p k t", k=KT))

                # ---- q from own x_ln: out [1024, TOK] ----
                wq = [wbig.tile([P, 3 * D], BD, tag="w", name=f"wq{l}_{i}")
                      for i in range(KT)]
                for kk in range(KT):
                    nc.sync.dma_start(
                        out=wq[kk],
                        in_=wqkv[l * D + kk * P:l * D + (kk + 1) * P, :])
                qT = qtp.tile([P, KT, TOK], BD)
                for m in range(KT):
                    pq_ = pmm.tile([P, SB], F32, tag="pmm")
                    for kk in range(KT):
                        nc.tensor.matmul(
                            pq_[:, :TOK],
                            wq[kk][:, m * P:(m + 1) * P],
                            x_ln[:, kk, :],
                            start=(kk == 0), stop=(kk == KT - 1))
                    nc.scalar.activation(qT[:, m, :], pq_[:, :TOK],
                                         AF.Identity,
                                         bias=lb[:, m:m + 1])
                # ---- k from gathered x_ag: out [1024, SB] ----
                kTf = ktp.tile([P, KT, SB], BD)
                for m in range(KT):
                    pk_ = pmm.tile([P, SB], F32, tag="pmm")
                    for kk in range(KT):
                        nc.tensor.matmul(
                            pk_,
                            wq[kk][:, D + m * P:D + (m + 1) * P],
                            x_ag[:, kk, :],
                            start=(kk == 0), stop=(kk == KT - 1))
                    nc.scalar.activation(kTf[:, m, :], pk_,
                                         AF.Identity,
                                         bias=lb[:, 8 + m:9 + m])
                # ---- v (token-major) from x_ag: out [SB, 1024] ----
                vrow = smp.tile([1, D], BD, tag="vrow", bufs=1)
                nc.sync.dma_start(out=vrow, in_=bvrow[l:l + 1, :])
                vf = vfp.tile([P, 4, 1040], BD)
                nc.vector.memset(vf, 1.0)
                for tp in range(4):
                    for nn in range(2):
                        pv = pmm.tile([P, SB], F32, tag="pmm")
                        for kk in range(KT):
                            nc.tensor.matmul(
                                pv,
                                x_ag[:, kk, tp * P:(tp + 1) * P],
                                wq[kk][:, 2 * D + nn * 512:
                                       2 * D + (nn + 1) * 512],
                                start=(kk == 0), stop=False)
                        nc.tensor.matmul(
                            pv, ones_row,
                            vrow[:, nn * 512:(nn + 1) * 512],
                            start=False, stop=True)
                        nc.scalar.copy(
                            vf[:, tp, :].rearrange(
                                "p (g x) -> p g x", x=65)[:, nn * 8:(nn + 1) * 8, 0:64],
                            pv.rearrange("p (g x) -> p g x", x=64))

                # ---- attention: 16 heads, own q ----
                o_sb = oop.tile([P, KT, TOK], BD)
                for hh in range(H):
                    kk_h = hh // 2
                    po_ = (hh % 2) * 64
                    pscs = []
                    for cc in range(4):
                        pt = pscp.tile([P, TOK], F32, tag="psc")
                        nc.tensor.matmul(
                            pt,
                            kTf[po_:po_ + 64, kk_h, cc * P:(cc + 1) * P],
                            qT[po_:po_ + 64, kk_h, :],
                            start=True, stop=True)
                        pscs.append(pt)
                    ets = []
                    for cc in range(4):
                        er = exp_pool.tile([P, TOK], BD, tag="ex")
                        nc.scalar.activation(er, pscs[cc], AF.Exp,
                                             scale=SCALE)
                        nc.vector.tensor_mul(er, er, mask_sb[:, cc, :])
                        ets.append(er)
                    pav = pop.tile([65, TOK], F32, tag="po")
                    for cc in range(4):
                        nc.tensor.matmul(
                            pav,
                            vf[:, cc, hh * 65:(hh + 1) * 65],
                            ets[cc],
                            start=(cc == 0), stop=(cc == 3))
                    inv_f = smp.tile([1, TOK], F32, tag="sm")
                    nc.vector.reciprocal(inv_f, pav[64:65, :])
                    inv_b = smp.tile([1, TOK], BD, tag="smb")
                    nc.vector.tensor_copy(out=inv_b, in_=inv_f)
                    pbc = pop.tile([64, TOK], F32, tag="po")
                    nc.tensor.matmul(pbc, ones_row[:, 0:64], inv_b,
                                     start=True, stop=True)
                    o_un = exp_pool.tile([64, TOK], BD, tag="ex")
                    nc.scalar.copy(o_un, pav[0:64, :])
                    ib_sb = exp_pool.tile([64, TOK], BD, tag="ex")
                    nc.scalar.copy(ib_sb, pbc)
                    nc.vector.tensor_mul(o_sb[po_:po_ + 64, kk_h, :],
                                         o_un, ib_sb)

                # ---- proj + residual: h_b = h_a + proj(o) + bias ----
                wp = [wbig.tile([P, 4, D], BD, tag="w", name=f"wp{l}_{i}")
                      for i in range(2)]
                for g in range(2):
                    nc.sync.dma_start(
                        out=wp[g],
                        in_=wproj[l * D + g * 4 * P:l * D + (g + 1) * 4 * P, :]
                        .rearrange("(k p) c -> p k c", k=4))
                for m in range(KT):
                    pp = pmm.tile([P, SB], F32, tag="pmm")
                    for kk in range(KT):
                        nc.tensor.matmul(
                            pp[:, :TOK],
                            wp[kk // 4][:, kk % 4, m * P:(m + 1) * P],
                            o_sb[:, kk, :],
                            start=(kk == 0), stop=(kk == KT - 1))
                    nc.vector.scalar_tensor_tensor(
                        out=h_b[:, m, :],
                        in0=pp[:, :TOK],
                        scalar=lb[:, 24 + m:25 + m],
                        in1=h_a[:, m, :],
                        op0=OP.add, op1=OP.add)

                # ---- LN2 + FC + gelu ----
                x_b2 = xbp.tile([P, KT, TOK], BD)
                nc.scalar.copy(x_b2, h_b)
                x_ln2 = xlnp.tile([P, KT, TOK], BD)
                emit_ln(x_b2, TOK, x_ln2)
                wf = [wbig.tile([P, FF], BD, tag="w", name=f"wf{l}_{i}")
                      for i in range(KT)]
                for kk in range(KT):
                    nc.sync.dma_start(
                        out=wf[kk],
                        in_=wfc[l * D + kk * P:l * D + (kk + 1) * P, :])
                g_sb = ggp.tile([P, FFT, TOK], BD)
                for m in range(FFT):
                    pf = pmm.tile([P, SB], F32, tag="pmm")
                    for kk in range(KT):
                        nc.tensor.matmul(
                            pf[:, :TOK],
                            wf[kk][:, m * P:(m + 1) * P],
                            x_ln2[:, kk, :],
                            start=(kk == 0), stop=(kk == KT - 1))
                    nc.scalar.activation(g_sb[:, m, :], pf[:, :TOK],
                                         AF.Gelu_apprx_tanh,
                                         bias=lb[:, 32 + m:33 + m])
                # ---- MLP + residual: h_a = h_b + mlp(g) + bias ----
                wm = [wbig.tile([P, 4, D], BD, tag="w", name=f"wm{l}_{i}")
                      for i in range(8)]
                for g in range(8):
                    nc.sync.dma_start(
                        out=wm[g],
                        in_=wmlp[l * FF + g * 4 * P:l * FF + (g + 1) * 4 * P, :]
                        .rearrange("(k p) c -> p k c", k=4))
                for m in range(KT):
                    pm_ = pmm.tile([P, SB], F32, tag="pmm")
                    for kk in range(FFT):
                        nc.tensor.matmul(
                            pm_[:, :TOK],
                            wm[kk // 4][:, kk % 4, m * P:(m + 1) * P],
                            g_sb[:, kk, :],
                            start=(kk == 0), stop=(kk == FFT - 1))
                    nc.vector.scalar_tensor_tensor(
                        out=h_a[:, m, :],
                        in0=pm_[:, :TOK],
                        scalar=lb[:, 64 + m:65 + m],
                        in1=h_b[:, m, :],
                        op0=OP.add, op1=OP.add)

            # ---- final LN + lm_head ----
            x_bf = xbp.tile([P, KT, TOK], BD)
            nc.scalar.copy(x_bf, h_a)
            x_lnf = xlnp.tile([P, KT, TOK], BD)
            emit_ln(x_bf, TOK, x_lnf)
            for nn in range(NV):
                wh = wbig.tile([P, KT, 512], BD, tag="w")
                nc.sync.dma_start(
                    out=wh,
                    in_=whead[nn * KT * P:(nn + 1) * KT * P, :]
                    .rearrange("(k p) c -> p k c", k=KT))
                for tp in range(2):
                    ph = pmm.tile([P, SB], F32, tag="pmm")
                    for kk in range(KT):
                        nc.tensor.matmul(
                            ph,
                            x_lnf[:, kk, tp * P:(tp + 1) * P],
                            wh[:, kk, :],
                            start=(kk == 0), stop=(kk == KT - 1))
                    ob = obp.tile([P, 512], F32)
                    nc.scalar.copy(ob, ph)
                    nc.sync.dma_start(
                        out=out[tp * P:(tp + 1) * P,
                                nn * 512:(nn + 1) * 512],
                        in_=ob)

    nc.compile()
    return nc


_CACHE = {}


def _get_nc(n_layers):
    if n_layers not in _CACHE:
        _CACHE[n_layers] = _build(n_layers)
    return _CACHE[n_layers]


def _prep_host(inputs, n_layers):
    """Host-side: embeddings, LN-affine folding, layouts, per-core shards."""
    ids = np.asarray(inputs["input_ids"])
    tts = np.asarray(inputs["token_type_ids"])
    wte = np.asarray(inputs["wte"], np.float32)
    wtte = np.asarray(inputs["wtte"], np.float32)
    wpe = np.asarray(inputs["wpe"], np.float32)

    h0 = wte[ids] + wpe[None, :, :] + wtte[tts]          # [B, S, D]

    ln1_w = np.asarray(inputs["ln1_w"], np.float32)
    ln1_b = np.asarray(inputs["ln1_b"], np.float32)
    attn_w = np.asarray(inputs["attn_w"], np.float32)
    attn_b = np.asarray(inputs["attn_b"], np.float32)
    atp_w = np.asarray(inputs["atp_w"], np.float32)
    atp_b = np.asarray(inputs["atp_b"], np.float32)
    ln2_w = np.asarray(inputs["ln2_w"], np.float32)
    ln2_b = np.asarray(inputs["ln2_b"], np.float32)
    fc_w = np.asarray(inputs["fc_w"], np.float32)
    fc_b = np.asarray(inputs["fc_b"], np.float32)
    mlp_w = np.asarray(inputs["mlp_w"], np.float32)
    mlp_b = np.asarray(inputs["mlp_b"], np.float32)
    lnf_w = np.asarray(inputs["lnf_w"], np.float32)
    lnf_b = np.asarray(inputs["lnf_b"], np.float32)
    head_w = np.asarray(inputs["head_w"], np.float32)
    head_b = np.asarray(inputs["head_b"], np.float32)

    nl = n_layers
    wqkv = np.empty((nl * D, 3 * D), BF)
    wproj_ = np.empty((nl * D, D), BF)
    wfc_ = np.empty((nl * D, FF), BF)
    wmlp_ = np.empty((nl * FF, D), BF)
    bvec = np.zeros((nl * P, 72), np.float32)
    bvrow = np.zeros((nl, D), BF)
    for l in range(nl):
        wq = attn_w[l] * ln1_w[l][:, None]
        bq = attn_b[l] + ln1_b[l] @ attn_w[l]            # [3072]
        wqkv[l * D:(l + 1) * D] = wq.astype(BF)
        wproj_[l * D:(l + 1) * D] = atp_w[l].astype(BF)
        wfc_[l * D:(l + 1) * D] = (fc_w[l] * ln2_w[l][:, None]).astype(BF)
        wmlp_[l * FF:(l + 1) * FF] = mlp_w[l].astype(BF)
        # feature-major outputs use per-partition bias tiles
        bvec[l * P:(l + 1) * P, 0:8] = bq[0:D].reshape(8, P).T
        bvec[l * P:(l + 1) * P, 8:16] = bq[D:2 * D].reshape(8, P).T
        bvec[l * P:(l + 1) * P, 24:32] = atp_b[l].reshape(8, P).T
        bfc = fc_b[l] + ln2_b[l] @ fc_w[l]
        bvec[l * P:(l + 1) * P, 32:64] = bfc.reshape(32, P).T
        bvec[l * P:(l + 1) * P, 64:72] = mlp_b[l].reshape(8, P).T
        bvrow[l] = bq[2 * D:3 * D].astype(BF)            # v bias as row

    whf = (head_w * lnf_w[:, None]).astype(np.float32)
    whp = np.zeros((D, VPAD), np.float32)
    whp[:, :V] = whf
    # layout [NV, KT, P, 512] flattened
    whead = np.ascontiguousarray(
        whp.reshape(KT, P, NV, 512).transpose(2, 0, 1, 3)
    ).reshape(NV * KT * P, 512).astype(BF)
    bhost = lnf_b @ head_w + head_b                      # [V]

    in_maps = []
    for c in range(8):
        rho = c % 2
        batch = c // 2
        qb = QBLOCKS[rho]
        h0T = np.ascontiguousarray(
            np.concatenate(
                [h0[batch, qb[0] * P:(qb[0] + 1) * P],
                 h0[batch, qb[1] * P:(qb[1] + 1) * P]], axis=0).T
        ).astype(np.float32)                              # [D, TOK]
        mk = np.zeros((4 * P, TOK), BF)
        for cc in range(4):
            kb = BB[cc]
            for qh in range(2):
                qblk = qb[qh]
                kg = kb * P + np.arange(P)[:, None]
                qg = qblk * P + np.arange(P)[None, :]
                mk[cc * P:(cc + 1) * P, qh * P:(qh + 1) * P] = \
                    (kg <= qg).astype(BF)
        in_maps.append({
            "h0T": h0T,
            "wqkv": wqkv, "wproj": wproj_, "wfc": wfc_, "wmlp": wmlp_,
            "whead": whead, "bvec": bvec, "bvrow": bvrow,
            "masks": mk,
        })
    return in_maps, bhost


def kernel(**inputs):
    from concourse import bass_utils

    n_layers = N_LAYERS
    nc = _get_nc(n_layers)
    in_maps, bhost = _prep_host(inputs, n_layers)

    trace = bool(int(os.environ.get("GPT2_TRACE", "0")))
    res = bass_utils.run_bass_kernel_spmd(
        nc, in_maps, core_ids=list(range(8)), trace=trace)
    if trace:
        kernel.last_exec_time_ns = res.exec_time_ns
        kernel.last_results = res

    full = np.empty((B, S, V), np.float32)
    for c in range(8):
        o = res.results[c]["out"]                         # [TOK, VPAD]
        rho = c % 2
        batch = c // 2
        qb = QBLOCKS[rho]
        full[batch, qb[0] * P:(qb[0] + 1) * P] = o[0:P, :V]
        full[batch, qb[1] * P:(qb[1] + 1) * P] = o[P:2 * P, :V]
    full += bhost[None, None, :]
    return full



# revision 16
# speedup vs baseline: 8.9322x; 8.9322x over previous
"""GPT2 (L=12, D=1024, H=16, S=512, B=4, V=16386) on 8 trn2 NeuronCores.

Scheme: token-data-parallel. Each core owns 256 tokens (2 causal-balanced
blocks of 128 within one batch; pair cores 2c/2c+1 split batch c).
Per layer: LN1(own) -> pair-AllGather of x_ln (bf16) -> k/v for the whole
batch computed locally from the gathered x_ln -> attention for own q tokens
(causality via per-core 0/1 mask inputs, keeps the SPMD program uniform)
-> proj/LN2/FC/gelu/MLP on own tokens only. lm_head token-sharded.

v2: causal 25% skip (q-halves x kv-chunks), head-pair row-group packing,
deferred batched softmax division, approx reciprocals, bank-aligned PSUM
pools, v-bias folded into proj bias, bf16 logits DMA.
"""

import os
import numpy as np
import ml_dtypes

# ---- static config (must match reference.py) ----
L = 12
D = 1024
H = 16
DH = 64
S = 512
B = 4
V = 16386
EPS = 1e-5
SCALE = 1.0 / 8.0  # 1/sqrt(DH)

P = 128
KT = D // P           # 8 k-tiles over D
TOK = 256             # own tokens per core
SB = 512              # batch tokens (kv length)
FF = 4096
FFT = FF // P         # 32
VPAD = 16896          # 33 * 512
NV = VPAD // 512      # 33

BF = ml_dtypes.bfloat16

# rank-order kv column blocks: chunk cc -> seq block id
BB = [0, 3, 1, 2]
# core parity -> owned q blocks
QBLOCKS = {0: (0, 3), 1: (1, 2)}

N_LAYERS = int(os.environ.get("GPT2_N_LAYERS", str(L)))


def _build(n_layers):
    from concourse import bacc, bass, mybir
    import concourse.tile as tile

    F32 = mybir.dt.float32
    BD = mybir.dt.bfloat16
    AF = mybir.ActivationFunctionType
    OP = mybir.AluOpType

    nc = bacc.Bacc("TRN2", target_bir_lowering=False, debug=False,
                   num_devices=8)

    # ---- kernel I/O ----
    h0T = nc.dram_tensor("h0T", [D, TOK], F32, kind="ExternalInput").ap()
    wqkv = nc.dram_tensor("wqkv", [n_layers * D, 3 * D], BD,
                          kind="ExternalInput").ap()
    wproj = nc.dram_tensor("wproj", [n_layers * D, D], BD,
                           kind="ExternalInput").ap()
    wfc = nc.dram_tensor("wfc", [n_layers * D, FF], BD,
                         kind="ExternalInput").ap()
    wmlp = nc.dram_tensor("wmlp", [n_layers * FF, D], BD,
                          kind="ExternalInput").ap()
    whead = nc.dram_tensor("whead", [NV * KT * P, 512], BD,
                           kind="ExternalInput").ap()
    # biases per layer, laid out [n_layers*128, 72]:
    #   cols 0:8 q (8 ptiles), 8:16 k, 24:32 proj(+v@proj), 32:64 fc, 64:72 mlp
    bvec = nc.dram_tensor("bvec", [n_layers * P, 72], F32,
                          kind="ExternalInput").ap()
    # causal masks, per core: [128, 768] = [T1 512 | T2 256]
    masks = nc.dram_tensor("masks", [P, 768], BD,
                           kind="ExternalInput").ap()
    # head-pair broadcast selector [128, 8*128]
    selb = nc.dram_tensor("selb", [P, KT * P], BD,
                          kind="ExternalInput").ap()
    out = nc.dram_tensor("out", [TOK, VPAD], BD, kind="ExternalOutput").ap()

    # internal DRAM for the per-layer pair all-gather
    agin = []
    agout = []
    for l in range(n_layers):
        agin.append(nc.dram_tensor(f"agin{l}", [P, KT * TOK], BD,
                                   kind="Internal").ap())
        agout.append(nc.dram_tensor(f"agout{l}", [2 * P, KT * TOK], BD,
                                    kind="Internal").ap())

    from contextlib import ExitStack

    with tile.TileContext(nc) as tc:
        with ExitStack() as ctx:
            consts = ctx.enter_context(tc.tile_pool(name="consts", bufs=1))
            resid = ctx.enter_context(tc.tile_pool(name="resid", bufs=1))
            acts = ctx.enter_context(tc.tile_pool(name="acts", bufs=1))
            wqp = ctx.enter_context(tc.tile_pool(name="wqp", bufs=8))
            wpp = ctx.enter_context(tc.tile_pool(name="wpp", bufs=2))
            wfmp = ctx.enter_context(tc.tile_pool(name="wfmp", bufs=8))
            scr8 = ctx.enter_context(tc.tile_pool(name="scr8", bufs=2))
            exp_pool = ctx.enter_context(tc.tile_pool(name="ex", bufs=3))
            smp = ctx.enter_context(tc.tile_pool(name="sm", bufs=2))
            # PSUM: 3 + 2 + 1 + 2 = 8 banks
            pbig = ctx.enter_context(
                tc.tile_pool(name="pbig", bufs=2, space="PSUM"))
            psc1 = ctx.enter_context(
                tc.tile_pool(name="psc1", bufs=2, space="PSUM"))
            psc2 = ctx.enter_context(
                tc.tile_pool(name="psc2", bufs=2, space="PSUM"))
            pavp = ctx.enter_context(
                tc.tile_pool(name="pavp", bufs=2, space="PSUM"))

            # ---- constants ----
            ones_col = consts.tile([P, 1], BD)
            nc.vector.memset(ones_col, 1.0)
            ones_row = consts.tile([1, P], BD)
            nc.vector.memset(ones_row, 1.0)
            eps_sb = consts.tile([1, 1], F32)
            nc.vector.memset(eps_sb, EPS)
            mask_sb = consts.tile([P, 768], BD)
            nc.sync.dma_start(out=mask_sb, in_=masks)
            sel_sb = consts.tile([P, KT, P], BD)
            nc.sync.dma_start(out=sel_sb,
                              in_=selb.rearrange("r (k p) -> r k p", k=KT))
            ball = consts.tile([P, n_layers, 72], F32)
            nc.sync.dma_start(
                out=ball,
                in_=bvec.rearrange("(l p) j -> p l j", l=n_layers))

            # persistent activations (o_un doubles as LN x^2 scratch)
            h_a = resid.tile([P, KT, TOK], F32)
            h_b = resid.tile([P, KT, TOK], F32)
            x_ln = acts.tile([P, KT, TOK], BD)
            qT = acts.tile([P, KT, TOK], BD)
            kTf = acts.tile([P, KT, SB], BD)
            vf = acts.tile([P, 4, 1040], BD)
            o_un = acts.tile([P, KT, TOK], BD)
            sq = o_un
            # head h denom at partition (h%4)*32, free slot h//4
            dn_all = acts.tile([P, 4, TOK], F32)
            inv_b = acts.tile([P, 4, TOK], BD)
            ms2 = acts.tile([P, 2, TOK], BD)

            nc.vector.memset(vf, 1.0)  # ones cols persist across layers
            nc.vector.memset(dn_all, 1.0)  # unused lanes stay finite
            nc.sync.dma_start(out=h_a,
                              in_=h0T.rearrange("(k p) t -> p k t", k=KT))

            def emit_ln(h_src):
                """h_src f32 [P,KT,TOK] -> x_ln (normalized, bf16)."""
                nc.scalar.copy(x_ln, h_src)                    # cast
                nc.vector.tensor_mul(sq, x_ln, x_ln)
                st = pbig.tile([1, 512], F32, tag="pbig",
                               padded_shape=[1, 512])
                for kk in range(KT):
                    nc.tensor.matmul(st[:, 0:TOK], ones_col, x_ln[:, kk, :],
                                     start=(kk == 0), stop=False,
                                     skip_group_check=True)
                for kk in range(KT):
                    nc.tensor.matmul(st[:, TOK:512], ones_col, sq[:, kk, :],
                                     start=False, stop=(kk == KT - 1),
                                     skip_group_check=True)
                mu = smp.tile([1, TOK], F32, tag="sm")
                t = smp.tile([1, TOK], F32, tag="sm")
                mu_b = smp.tile([1, TOK], BD, tag="smb")
                iv_b = smp.tile([1, TOK], BD, tag="smb")
                nc.vector.tensor_scalar_mul(mu, st[:, 0:TOK], 1.0 / D)
                nc.vector.tensor_copy(out=mu_b, in_=mu)
                nc.vector.scalar_tensor_tensor(
                    out=t, in0=st[:, 0:TOK], scalar=1.0 / D, in1=mu,
                    op0=OP.mult, op1=OP.mult)
                nc.vector.scalar_tensor_tensor(
                    out=t, in0=st[:, TOK:512], scalar=1.0 / D, in1=t,
                    op0=OP.mult, op1=OP.subtract)
                nc.scalar.activation(t, t, AF.Sqrt, bias=eps_sb)
                nc.vector.reciprocal_approx_fast(out=t, in_=t)
                nc.vector.tensor_copy(out=iv_b, in_=t)
                pb2 = pbig.tile([P, 512], F32, tag="pbig")
                nc.tensor.matmul(pb2[:, 0:TOK], ones_row, mu_b,
                                 start=True, stop=False,
                                 skip_group_check=True)
                nc.tensor.matmul(pb2[:, TOK:512], ones_row, iv_b,
                                 start=False, stop=True,
                                 skip_group_check=True)
                nc.scalar.copy(ms2, pb2.rearrange("p (i t) -> p i t", i=2))
                for kk in range(KT):
                    nc.vector.tensor_sub(x_ln[:, kk, :], x_ln[:, kk, :],
                                         ms2[:, 0, :])
                    nc.vector.tensor_mul(x_ln[:, kk, :], x_ln[:, kk, :],
                                         ms2[:, 1, :])

            for l in range(n_layers):
                lb = ball[:, l, :]
                # ---- LN1 on own tokens ----
                emit_ln(h_a)

                # ---- all-gather x_ln across the pair ----
                nc.sync.dma_start(
                    out=agin[l].rearrange("p (k t) -> p k t", k=KT),
                    in_=x_ln)
                nc.gpsimd.collective_compute(
                    "AllGather", mybir.AluOpType.bypass,
                    replica_groups=[[0, 1], [2, 3], [4, 5], [6, 7]],
                    ins=[agin[l]],
                    outs=[agout[l]],
                )

                # ---- q from own x_ln: out [1024, TOK] ----
                wq = [wqp.tile([P, 3 * D], BD, tag="wq", name=f"wq{l}_{i}")
                      for i in range(KT)]
                for kk in range(KT):
                    nc.sync.dma_start(
                        out=wq[kk],
                        in_=wqkv[l * D + kk * P:l * D + (kk + 1) * P, :])
                for m in range(KT):
                    pq_ = pbig.tile([P, 512], F32, tag="pbig")
                    for kk in range(KT):
                        nc.tensor.matmul(
                            pq_[:, :TOK],
                            wq[kk][:, m * P:(m + 1) * P],
                            x_ln[:, kk, :],
                            start=(kk == 0), stop=(kk == KT - 1))
                    nc.scalar.activation(qT[:, m, :], pq_[:, :TOK],
                                         AF.Identity,
                                         bias=lb[:, m:m + 1])

                # gathered x_ln for k/v
                x_ag = scr8.tile([P, KT, SB], BD, tag="scr8",
                                 name=f"xag{l}")
                for s in range(2):
                    nc.sync.dma_start(
                        out=x_ag[:, :, s * TOK:(s + 1) * TOK],
                        in_=agout[l][s * P:(s + 1) * P, :].rearrange(
                            "p (k t) -> p k t", k=KT))

                # ---- k from gathered x_ag: out [1024, SB] ----
                for m in range(KT):
                    pk_ = pbig.tile([P, 512], F32, tag="pbig")
                    for kk in range(KT):
                        nc.tensor.matmul(
                            pk_,
                            wq[kk][:, D + m * P:D + (m + 1) * P],
                            x_ag[:, kk, :],
                            start=(kk == 0), stop=(kk == KT - 1))
                    nc.scalar.activation(kTf[:, m, :], pk_,
                                         AF.Identity,
                                         bias=lb[:, 8 + m:9 + m])
                # ---- v (token-major) from x_ag (no bias; folded in proj) ----
                for tp in range(4):
                    for nn in range(2):
                        pv = pbig.tile([P, 512], F32, tag="pbig")
                        for kk in range(KT):
                            nc.tensor.matmul(
                                pv,
                                x_ag[:, kk, tp * P:(tp + 1) * P],
                                wq[kk][:, 2 * D + nn * 512:
                                       2 * D + (nn + 1) * 512],
                                start=(kk == 0), stop=(kk == KT - 1))
                        nc.scalar.copy(
                            vf[:, tp, :].rearrange(
                                "p (g x) -> p g x", x=65)[:, nn * 8:(nn + 1) * 8, 0:64],
                            pv.rearrange("p (g x) -> p g x", x=64))

                # ---- attention: head pairs, 25% causal skip ----
                # T1 [128,512] = cc0 over q0:256 | cc2 over q0:256
                # T2 [128,256] = cc1 over q128:256 | cc3 over q128:256
                for j in range(KT):
                    t1 = [psc1.tile([P, 512], F32, tag="t1",
                                    name=f"t1_{l}_{j}_{i}")
                          for i in range(2)]
                    t2 = [psc2.tile([P, TOK], F32, tag="t2",
                                    name=f"t2_{l}_{j}_{i}")
                          for i in range(2)]
                    for cc, dst, dof in ((0, t1, 0), (2, t1, TOK)):
                        for i in range(2):
                            po_ = i * 64
                            nc.tensor.matmul(
                                dst[i][:, dof:dof + TOK],
                                kTf[po_:po_ + 64, j, cc * P:(cc + 1) * P],
                                qT[po_:po_ + 64, j, :],
                                start=True, stop=True)
                    for cc, dst, dof in ((1, t2, 0), (3, t2, P)):
                        for i in range(2):
                            po_ = i * 64
                            nc.tensor.matmul(
                                dst[i][:, dof:dof + P],
                                kTf[po_:po_ + 64, j, cc * P:(cc + 1) * P],
                                qT[po_:po_ + 64, j, P:TOK],
                                start=True, stop=True)
                    e1 = [exp_pool.tile([P, 512], BD, tag="e1",
                                        name=f"e1_{l}_{j}_{i}")
                          for i in range(2)]
                    e2 = [exp_pool.tile([P, TOK], BD, tag="e2",
                                        name=f"e2_{l}_{j}_{i}")
                          for i in range(2)]
                    for i in range(2):
                        nc.scalar.activation(e1[i], t1[i], AF.Exp,
                                             scale=SCALE)
                        nc.scalar.activation(e2[i], t2[i], AF.Exp,
                                             scale=SCALE)
                        nc.vector.tensor_mul(e1[i], e1[i],
                                             mask_sb[:, 0:512])
                        nc.vector.tensor_mul(e2[i], e2[i],
                                             mask_sb[:, 512:768])
                    for i in range(2):
                        hh = 2 * j + i
                        pav = pavp.tile([65, TOK], F32, tag="pav",
                                        padded_shape=[65, 512],
                                        name=f"pav_{l}_{j}_{i}")
                        nc.tensor.matmul(
                            pav, vf[:, 0, hh * 65:(hh + 1) * 65],
                            e1[i][:, 0:TOK],
                            start=True, stop=False, skip_group_check=True)
                        nc.tensor.matmul(
                            pav, vf[:, 2, hh * 65:(hh + 1) * 65],
                            e1[i][:, TOK:512],
                            start=False, stop=False, skip_group_check=True)
                        nc.tensor.matmul(
                            pav[:, P:TOK], vf[:, 1, hh * 65:(hh + 1) * 65],
                            e2[i][:, 0:P],
                            start=False, stop=False, skip_group_check=True)
                        nc.tensor.matmul(
                            pav[:, P:TOK], vf[:, 3, hh * 65:(hh + 1) * 65],
                            e2[i][:, P:TOK],
                            start=False, stop=True, skip_group_check=True)
                        po_ = i * 64
                        nc.scalar.copy(o_un[po_:po_ + 64, j, :],
                                       pav[0:64, :])
                        dp = (hh % 4) * 32
                        if i == 0:
                            nc.vector.tensor_copy(
                                out=dn_all[dp:dp + 1, hh // 4, :],
                                in_=pav[64:65, :])
                        else:
                            nc.scalar.copy(dn_all[dp:dp + 1, hh // 4, :],
                                           pav[64:65, :])

                # deferred softmax division, batched over all 16 heads
                nc.vector.reciprocal_approx_fast(out=dn_all, in_=dn_all)
                nc.vector.tensor_copy(out=inv_b, in_=dn_all)
                for j in range(KT):
                    pbc = pavp.tile([P, TOK], F32, tag="pav",
                                    padded_shape=[P, 512],
                                    name=f"pbc_{l}_{j}")
                    nc.tensor.matmul(pbc, sel_sb[:, j, :],
                                     inv_b[:, j // 2, :],
                                     start=True, stop=True)
                    nc.vector.tensor_mul(o_un[:, j, :], o_un[:, j, :],
                                         pbc)

                # ---- proj + residual: h_b = h_a + proj(o) + bias ----
                wp = [wpp.tile([P, 4, D], BD, tag="wp", name=f"wp{l}_{i}")
                      for i in range(2)]
                for g in range(2):
                    nc.sync.dma_start(
                        out=wp[g],
                        in_=wproj[l * D + g * 4 * P:l * D + (g + 1) * 4 * P, :]
                        .rearrange("(k p) c -> p k c", k=4))
                for m in range(KT):
                    pp = pbig.tile([P, 512], F32, tag="pbig")
                    for kk in range(KT):
                        nc.tensor.matmul(
                            pp[:, :TOK],
                            wp[kk // 4][:, kk % 4, m * P:(m + 1) * P],
                            o_un[:, kk, :],
                            start=(kk == 0), stop=(kk == KT - 1))
                    nc.vector.scalar_tensor_tensor(
                        out=h_b[:, m, :],
                        in0=pp[:, :TOK],
                        scalar=lb[:, 24 + m:25 + m],
                        in1=h_a[:, m, :],
                        op0=OP.add, op1=OP.add)

                # ---- LN2 + FC + gelu ----
                emit_ln(h_b)
                wf = [wfmp.tile([P, FF], BD, tag="wfm", name=f"wf{l}_{i}")
                      for i in range(KT)]
                for kk in range(KT):
                    nc.sync.dma_start(
                        out=wf[kk],
                        in_=wfc[l * D + kk * P:l * D + (kk + 1) * P, :])
                g_sb = [scr8.tile([P, FFT // 2, TOK], BD, tag="scr8",
                                  name=f"g{l}_{i}")
                        for i in range(2)]
                for m in range(FFT):
                    pf = pbig.tile([P, 512], F32, tag="pbig")
                    for kk in range(KT):
                        nc.tensor.matmul(
                            pf[:, :TOK],
                            wf[kk][:, m * P:(m + 1) * P],
                            x_ln[:, kk, :],
                            start=(kk == 0), stop=(kk == KT - 1))
                    nc.scalar.activation(g_sb[m // 16][:, m % 16, :],
                                         pf[:, :TOK],
                                         AF.Gelu_apprx_tanh,
                                         bias=lb[:, 32 + m:33 + m])
                # ---- MLP + residual: h_a = h_b + mlp(g) + bias ----
                wm = [wfmp.tile([P, 4, D], BD, tag="wfm", name=f"wm{l}_{i}")
                      for i in range(8)]
                for g in range(8):
                    nc.sync.dma_start(
                        out=wm[g],
                        in_=wmlp[l * FF + g * 4 * P:l * FF + (g + 1) * 4 * P, :]
                        .rearrange("(k p) c -> p k c", k=4))
                for m in range(KT):
                    pm_ = pbig.tile([P, 512], F32, tag="pbig")
                    for kk in range(FFT):
                        nc.tensor.matmul(
                            pm_[:, :TOK],
                            wm[kk // 4][:, kk % 4, m * P:(m + 1) * P],
                            g_sb[kk // 16][:, kk % 16, :],
                            start=(kk == 0), stop=(kk == FFT - 1))
                    nc.vector.scalar_tensor_tensor(
                        out=h_a[:, m, :],
                        in0=pm_[:, :TOK],
                        scalar=lb[:, 64 + m:65 + m],
                        in1=h_b[:, m, :],
                        op0=OP.add, op1=OP.add)

            # ---- final LN + lm_head ----
            emit_ln(h_a)
            for nn in range(NV):
                wh = wfmp.tile([P, KT, 512], BD, tag="wfm",
                               name=f"wh{nn}")
                nc.sync.dma_start(
                    out=wh,
                    in_=whead[nn * KT * P:(nn + 1) * KT * P, :]
                    .rearrange("(k p) c -> p k c", k=KT))
                for tp in range(2):
                    ph = pbig.tile([P, 512], F32, tag="pbig")
                    for kk in range(KT):
                        nc.tensor.matmul(
                            ph,
                            x_ln[:, kk, tp * P:(tp + 1) * P],
                            wh[:, kk, :],
                            start=(kk == 0), stop=(kk == KT - 1))
                    ob = exp_pool.tile([P, 512], BD, tag="e1",
                                       name=f"ob{nn}_{tp}")
                    if (nn * 2 + tp) % 2 == 0:
                        nc.scalar.copy(ob, ph)
                    else:
                        nc.vector.tensor_copy(out=ob, in_=ph)
                    nc.sync.dma_start(
                        out=out[tp * P:(tp + 1) * P,
                                nn * 512:(nn + 1) * 512],
                        in_=ob)

    nc.compile()
    return nc


_CACHE = {}


def _get_nc(n_layers):
    if n_layers not in _CACHE:
        _CACHE[n_layers] = _build(n_layers)
    return _CACHE[n_layers]


def _prep_host(inputs, n_layers):
    """Host-side: embeddings, LN-affine folding, layouts, per-core shards."""
    ids = np.asarray(inputs["input_ids"])
    tts = np.asarray(inputs["token_type_ids"])
    wte = np.asarray(inputs["wte"], np.float32)
    wtte = np.asarray(inputs["wtte"], np.float32)
    wpe = np.asarray(inputs["wpe"], np.float32)

    h0 = wte[ids] + wpe[None, :, :] + wtte[tts]          # [B, S, D]

    ln1_w = np.asarray(inputs["ln1_w"], np.float32)
    ln1_b = np.asarray(inputs["ln1_b"], np.float32)
    attn_w = np.asarray(inputs["attn_w"], np.float32)
    attn_b = np.asarray(inputs["attn_b"], np.float32)
    atp_w = np.asarray(inputs["atp_w"], np.float32)
    atp_b = np.asarray(inputs["atp_b"], np.float32)
    ln2_w = np.asarray(inputs["ln2_w"], np.float32)
    ln2_b = np.asarray(inputs["ln2_b"], np.float32)
    fc_w = np.asarray(inputs["fc_w"], np.float32)
    fc_b = np.asarray(inputs["fc_b"], np.float32)
    mlp_w = np.asarray(inputs["mlp_w"], np.float32)
    mlp_b = np.asarray(inputs["mlp_b"], np.float32)
    lnf_w = np.asarray(inputs["lnf_w"], np.float32)
    lnf_b = np.asarray(inputs["lnf_b"], np.float32)
    head_w = np.asarray(inputs["head_w"], np.float32)
    head_b = np.asarray(inputs["head_b"], np.float32)

    nl = n_layers
    wqkv = np.empty((nl * D, 3 * D), BF)
    wproj_ = np.empty((nl * D, D), BF)
    wfc_ = np.empty((nl * D, FF), BF)
    wmlp_ = np.empty((nl * FF, D), BF)
    bvec = np.zeros((nl * P, 72), np.float32)
    for l in range(nl):
        wq = attn_w[l] * ln1_w[l][:, None]
        bq = attn_b[l] + ln1_b[l] @ attn_w[l]            # [3072]
        wqkv[l * D:(l + 1) * D] = wq.astype(BF)
        wproj_[l * D:(l + 1) * D] = atp_w[l].astype(BF)
        wfc_[l * D:(l + 1) * D] = (fc_w[l] * ln2_w[l][:, None]).astype(BF)
        wmlp_[l * FF:(l + 1) * FF] = mlp_w[l].astype(BF)
        # feature-major outputs use per-partition bias tiles
        bvec[l * P:(l + 1) * P, 0:8] = bq[0:D].reshape(8, P).T
        bvec[l * P:(l + 1) * P, 8:16] = bq[D:2 * D].reshape(8, P).T
        # v bias folded through proj: bp' = bp + bv @ Wp
        bp = atp_b[l] + bq[2 * D:3 * D] @ atp_w[l]
        bvec[l * P:(l + 1) * P, 24:32] = bp.reshape(8, P).T
        bfc = fc_b[l] + ln2_b[l] @ fc_w[l]
        bvec[l * P:(l + 1) * P, 32:64] = bfc.reshape(32, P).T
        bvec[l * P:(l + 1) * P, 64:72] = mlp_b[l].reshape(8, P).T

    whf = (head_w * lnf_w[:, None]).astype(np.float32)
    whp = np.zeros((D, VPAD), np.float32)
    whp[:, :V] = whf
    # layout [NV, KT, P, 512] flattened
    whead = np.ascontiguousarray(
        whp.reshape(KT, P, NV, 512).transpose(2, 0, 1, 3)
    ).reshape(NV * KT * P, 512).astype(BF)
    bhost = lnf_b @ head_w + head_b                      # [V]

    # head-pair broadcast selector [128, KT*P]: head h=2j+i lives at
    # inv_b partition (h%4)*32, slot h//4=j//2; pbc_j = sel[:,j,:].T @ inv_b
    selb = np.zeros((P, KT * P), BF)
    for j in range(KT):
        ha, hb = 2 * j, 2 * j + 1
        selb[(ha % 4) * 32, j * P:j * P + 64] = 1.0
        selb[(hb % 4) * 32, j * P + 64:(j + 1) * P] = 1.0

    in_maps = []
    for c in range(8):
        rho = c % 2
        batch = c // 2
        qb = QBLOCKS[rho]
        h0T = np.ascontiguousarray(
            np.concatenate(
                [h0[batch, qb[0] * P:(qb[0] + 1) * P],
                 h0[batch, qb[1] * P:(qb[1] + 1) * P]], axis=0).T
        ).astype(np.float32)                              # [D, TOK]
        # masks: T1 [128,512] = cc0 q0:256 | cc2 q0:256
        #        T2 [128,256] = cc1 q128:256 | cc3 q128:256
        mk = np.zeros((P, 768), BF)

        def blockmask(kb, qblk):
            kg = kb * P + np.arange(P)[:, None]
            qg = qblk * P + np.arange(P)[None, :]
            return (kg <= qg).astype(BF)

        for qh in range(2):
            mk[:, qh * P:(qh + 1) * P] = blockmask(BB[0], qb[qh])
            mk[:, 256 + qh * P:256 + (qh + 1) * P] = blockmask(BB[2], qb[qh])
        mk[:, 512:640] = blockmask(BB[1], qb[1])
        mk[:, 640:768] = blockmask(BB[3], qb[1])
        in_maps.append({
            "h0T": h0T,
            "wqkv": wqkv, "wproj": wproj_, "wfc": wfc_, "wmlp": wmlp_,
            "whead": whead, "bvec": bvec, "selb": selb,
            "masks": mk,
        })
    return in_maps, bhost


def kernel(**inputs):
    from concourse import bass_utils

    n_layers = N_LAYERS
    nc = _get_nc(n_layers)
    in_maps, bhost = _prep_host(inputs, n_layers)

    trace = bool(int(os.environ.get("GPT2_TRACE", "0")))
    res = bass_utils.run_bass_kernel_spmd(
        nc, in_maps, core_ids=list(range(8)), trace=trace)
    if trace:
        kernel.last_exec_time_ns = res.exec_time_ns
        kernel.last_results = res

    full = np.empty((B, S, V), np.float32)
    for c in range(8):
        o = np.asarray(res.results[c]["out"]).astype(np.float32)
        rho = c % 2
        batch = c // 2
        qb = QBLOCKS[rho]
        full[batch, qb[0] * P:(qb[0] + 1) * P] = o[0:P, :V]
        full[batch, qb[1] * P:(qb[1] + 1) * P] = o[P:2 * P, :V]
    full += bhost[None, None, :]
    return full
